# revision 1
# baseline (speedup 1.0000x reference)
"""EMAttention2d (vq_codebook) Trainium2 kernel.

Data parallel over batch: 16 images -> 8 cores x 2 images. Single kernel
launch per core; BN batch stats cross-core reduced with a tiny AllReduce.

Math (validated vs reference, fp32):
  per image, X = x[b] reshaped (C, N):
    mu_0 = mu
    repeat 3x:
      mutT = mu^T Ws            (K, C)      [stem folded into codebook]
      mub  = mu^T bs            (K,)
      A^T  = mutT X + mub       (K, N)
      E^T  = exp(A^T)                      [no max-sub: logits bounded]
      E    = transpose(E^T)     (N, K)
      s_n  = sum_k E            (N,)
      z    = E / s_n            (N, K)     [proper softmax]
      G    = z^T X^T            (K, C)
      s_k  = z^T 1              (K,)
      muRT = G Ws^T + s_k bs^T  (K, C)
      muT  = muRT / ||row||_2              [zn / 1e-6 normalizers cancel]
    y2   = mu3 z3^T             (C, N)
    G2   = Hw relu(y2)          (C, N)     [head bias drops out of BN]
  BN over batch of G2 (AllReduce of per-channel sum/sumsq), then
    out = relu(G2 * a + b2 + x),  a = gamma*rstd, b2 = beta - mean*a

SBUF phasing (pools are stack-scoped):
  L1 (whole kernel): consts, z (2 imgs), muT (2 imgs), small staging, psum
  L2 (transpose+EM of both imgs): X, X^T, exp staging   -- 134 KB/part
  L3 (y2/head/BN/final, opened after L2 closes): head weights, zT, ry2,
     h1 (img1 pre-BN acts, bf16, SBUF), h0 spill staging, final staging
"""

import sys

for _p in ("/opt/trn_rl_repo",):
    if _p not in sys.path:
        sys.path.insert(0, _p)

import numpy as np

B, C, N, K = 16, 512, 4096, 64
NCORES = 8
BPC = B // NCORES  # images per core
P = 128
OC = C // P   # 4 chunks of channels
NT = N // P   # 32 pixel tiles
NCH = N // 512  # 8 pixel chunks of 512
BN_EPS = 1e-5
NUM_ITER = 3

_cache = {}


def _build_nc(n_devices=NCORES, use_collective=True):
    import concourse.bass as bass
    import concourse.mybir as mybir
    import concourse.tile as tile
    from concourse.masks import make_identity
    from contextlib import ExitStack

    dt = mybir.dt
    f32 = dt.float32
    f32r = dt.float32r
    bf16 = dt.bfloat16
    AF = mybir.ActivationFunctionType
    ALU = mybir.AluOpType
    AX = mybir.AxisListType

    nc = bass.Bass("TRN2", target_bir_lowering=False, debug=False,
                   num_devices=n_devices)

    x_d = nc.dram_tensor("x", [BPC, C, N], f32, kind="ExternalInput").ap()
    mu_d = nc.dram_tensor("mu", [C, K], f32, kind="ExternalInput").ap()
    ws_d = nc.dram_tensor("ws", [C, C], f32, kind="ExternalInput").ap()
    wst_d = nc.dram_tensor("wst", [C, C], f32, kind="ExternalInput").ap()
    hwt_d = nc.dram_tensor("hwt", [C, C], f32, kind="ExternalInput").ap()
    bs_d = nc.dram_tensor("bs", [C, 2], f32, kind="ExternalInput").ap()
    gm_d = nc.dram_tensor("gm", [C], f32, kind="ExternalInput").ap()
    onec_d = nc.dram_tensor("onec", [P], f32, kind="ExternalInput").ap()
    bt_d = nc.dram_tensor("bt", [C], f32, kind="ExternalInput").ap()
    out_d = nc.dram_tensor("out", [BPC, C, N], f32, kind="ExternalOutput").ap()
    h0_d = nc.dram_tensor("h0spill", [C, N], bf16).ap()
    st_in_d = nc.dram_tensor("stats_in", [P, 2 * OC], f32).ap()
    st_out_d = nc.dram_tensor("stats_out", [P, 2 * OC], f32,
                              addr_space="Shared").ap()

    with tile.TileContext(nc) as tc, ExitStack() as ctx:
        consts = ctx.enter_context(tc.tile_pool(name="consts", bufs=1))
        zpool = ctx.enter_context(tc.tile_pool(name="zpool", bufs=2))
        mutp = ctx.enter_context(tc.tile_pool(name="mutp", bufs=4))
        munp = ctx.enter_context(tc.tile_pool(name="munp", bufs=2))
        smalls = ctx.enter_context(tc.tile_pool(name="smalls", bufs=2))
        statp = ctx.enter_context(tc.tile_pool(name="statp", bufs=1))

        # ---- constants ----
        id_sb = consts.tile([P, P], f32)
        make_identity(nc, id_sb[:])
        ws_sb = consts.tile([P, OC, C], f32r)    # Ws rows  (o_part, oc, ci)
        nc.sync.dma_start(ws_sb[:], ws_d.rearrange("(t p) c -> p t c", p=P).bitcast(f32r))
        wst_sb = consts.tile([P, OC, C], f32r)   # Ws^T rows (ci_part, cc, o)
        nc.sync.dma_start(wst_sb[:], wst_d.rearrange("(t p) c -> p t c", p=P).bitcast(f32r))
        mu0_sb = consts.tile([P, OC, K], f32r)
        nc.sync.dma_start(mu0_sb[:], mu_d.rearrange("(t p) k -> p t k", p=P).bitcast(f32r))
        b_sb = consts.tile([P, OC, 2], f32r)
        nc.sync.dma_start(b_sb[:],
                          bs_d.rearrange("(t p) two -> p t two", p=P)
                          .bitcast(f32r))
        bs_row = consts.tile([1, C], f32r)
        nc.sync.dma_start(bs_row[:], bs_d[:, 0][None, :].bitcast(f32r))
        gm_sb = consts.tile([P, OC], f32)
        nc.sync.dma_start(gm_sb[:], gm_d.rearrange("(t p) -> p t", p=P))
        bt_sb = consts.tile([P, OC], f32)
        nc.sync.dma_start(bt_sb[:], bt_d.rearrange("(t p) -> p t", p=P))
        ones_col = consts.tile([P, 1], f32r)
        nc.sync.dma_start(ones_col[:], onec_d[:, None].bitcast(f32r))
        eps_sb = consts.tile([P, 1], f32)
        nc.vector.memset(eps_sb[:], BN_EPS)
        idr = id_sb[:]  # fp32 transposes (f32r transpose trips walrus sync-wait limit)

        sum_acc = statp.tile([P, OC, BPC * NCH], f32)
        sq_acc = statp.tile([P, OC, BPC * NCH], f32)

        z_of = [None] * BPC
        muT_of = [None] * BPC

        # ================= L2: transpose + EM for both images ============
        with ExitStack() as l2:
            xpool = l2.enter_context(tc.tile_pool(name="xpool", bufs=16))
            xtpool = l2.enter_context(tc.tile_pool(name="xtpool", bufs=32))
            etpool = l2.enter_context(tc.tile_pool(name="etpool", bufs=3))
            psum2 = l2.enter_context(tc.tile_pool(name="psum2", bufs=1,
                                                  space="PSUM"))

            def ps(nm, bufs_tag=None):
                return psum2.tile([P, 512], f32, tag=bufs_tag or nm, name=nm)

            for b in range(BPC):
                # X as 16 quarter tiles (cc, q) so transposes start after
                # the first 2 MiB of the image load
                xh = {}
                for hf in range(4):
                    for cc in range(OC):
                        xc = xpool.tile([P, N // 4], f32r, tag="X",
                                        name=f"x{b}_{cc}_{hf}")
                        nc.sync.dma_start(
                            xc[:],
                            x_d[b, cc * P:(cc + 1) * P,
                                hf * (N // 4):(hf + 1) * (N // 4)]
                            .bitcast(f32r))
                        xh[(cc, hf)] = xc

                def xs(cc, col, width):
                    hf = col // (N // 4)
                    off = col - hf * (N // 4)
                    return xh[(cc, hf)][:, off:off + width]
                xt_sb = []
                for tt in range(NT):
                    xt = xtpool.tile([P, C], f32r, tag="xT", name=f"xt{b}_{tt}")
                    pst = ps("xtr", "xtr%d" % (tt % 2))
                    for cc in range(OC):
                        nc.tensor.transpose(
                            pst[:, cc * P:(cc + 1) * P],
                            xs(cc, tt * P, P).bitcast(f32),
                            idr)
                    if tt % 2 == 0:
                        nc.vector.tensor_copy(xt[:], pst[:])
                    else:
                        nc.scalar.copy(xt[:], pst[:])
                    xt_sb.append(xt)

                # ---- EM iterations ----
                mu_nat = mu0_sb  # (P, OC, K) natural layout of current mu
                z_sb = zpool.tile([P, NT, K], f32r, tag="z", name=f"z{b}")
                z_of[b] = z_sb
                for it in range(NUM_ITER):
                    # mu~^T = mu^T Ws (K, C); transpose to (ci, k) chunks
                    mutT_ps = ps("mm")
                    for oc in range(OC):
                        nc.tensor.matmul(mutT_ps[:K, :].bitcast(f32),
                                         mu_nat[:, oc, :],
                                         ws_sb[:, oc, :],
                                         start=(oc == 0), stop=(oc == OC - 1))
                    mutT_sb = smalls.tile([K, C], f32r, tag="kc")
                    nc.vector.tensor_copy(mutT_sb[:], mutT_ps[:K, :])
                    mut_ps = ps("mm2")
                    for cc in range(OC):
                        nc.tensor.transpose(
                            mut_ps[:, cc * K:(cc + 1) * K],
                            mutT_sb[:, cc * P:(cc + 1) * P].bitcast(f32),
                            idr[:K, :K])
                    mut_sb = smalls.tile([P, OC, K], f32r, tag="mut")
                    nc.vector.tensor_copy(mut_sb[:], mut_ps[:, :OC * K])

                    # mub = mu^T bs  (K, 1)
                    mub_ps = ps("mm2")
                    for oc in range(OC):
                        nc.tensor.matmul(mub_ps[:K, :2].bitcast(f32),
                                         mu_nat[:, oc, :],
                                         b_sb[:, oc, :],
                                         start=(oc == 0), stop=(oc == OC - 1))
                    mub_sb = smalls.tile([K, 1], f32, tag="mub")
                    nc.vector.tensor_copy(mub_sb[:], mub_ps[:K, :1])

                    # A^T chunks -> exp -> transpose -> z
                    def z_block(et, ch):
                        e_ps = ps("Etr", "Etr%d" % (ch % 2))
                        e3 = e_ps[:, :4 * K].rearrange("p (j k) -> p j k",
                                                       k=K)
                        for j in range(4):
                            nc.tensor.transpose(
                                e3[:, j, :],
                                et[:, j * P:(j + 1) * P].bitcast(f32),
                                idr[:K, :K])
                        s4 = smalls.tile([P, 4], f32, tag="s4")
                        nc.vector.tensor_reduce(s4[:], e3[:], axis=AX.X,
                                                op=ALU.add)
                        nc.vector.reciprocal(s4[:], s4[:])
                        nc.vector.tensor_tensor(
                            z_sb[:, ch * 4:(ch + 1) * 4, :], e3[:],
                            s4[:, :, None].to_broadcast((P, 4, K)), ALU.mult)

                    pend = None
                    for ch in range(NCH):
                        a_ps = ps("A", "A%d" % (ch % 2))
                        for cc in range(OC):
                            nc.tensor.matmul(
                                a_ps[:K, :].bitcast(f32),
                                mut_sb[:, cc, :],
                                xs(cc, ch * 512, 512),
                                start=(cc == 0), stop=(cc == OC - 1))
                        et = etpool.tile([K, 512], f32r, tag="ET")
                        nc.scalar.activation(et[:], a_ps[:K, :], AF.Exp,
                                             bias=mub_sb[:], scale=1.0)
                        if pend is not None:
                            z_block(*pend)
                        pend = (et, ch)
                    z_block(*pend)

                    # G = z^T X^T ; s_k = z^T 1  (accumulate over tiles)
                    G_ps = ps("G", "xtr0")
                    sk_ps = ps("sk", "mm2")
                    for tt in range(NT):
                        nc.tensor.matmul(G_ps[:K, :].bitcast(f32),
                                         z_sb[:, tt, :],
                                         xt_sb[tt][:],
                                         start=(tt == 0), stop=(tt == NT - 1))
                    for tq in range(NT // 4):
                        nc.tensor.matmul(
                            sk_ps[:1, :4 * K].bitcast(f32),
                            ones_col[:],
                            z_sb[:, 4 * tq:4 * (tq + 1), :],
                            start=(tq == 0), stop=(tq == NT // 4 - 1))
                    g_sb = smalls.tile([K, C], f32r, tag="kc")
                    nc.vector.tensor_copy(g_sb[:], G_ps[:K, :])
                    sk_sb = smalls.tile([1, K], f32r, tag="sk")
                    with nc.allow_low_precision(
                            reason="f32r is 32-bit; rounding to f32r grid"):
                        nc.vector.tensor_reduce(
                            sk_sb[:],
                            sk_ps[:1, :4 * K].rearrange("p (f k) -> p k f",
                                                        k=K),
                            axis=AX.X, op=ALU.add)
                    gt_ps = ps("mm")
                    g3 = gt_ps[:, :OC * K].rearrange("p (j k) -> p j k", k=K)
                    for cc in range(OC):
                        nc.tensor.transpose(
                            g3[:, cc, :],
                            g_sb[:, cc * P:(cc + 1) * P].bitcast(f32),
                            idr[:K, :K])
                    gt_sb = smalls.tile([P, OC, K], f32r, tag="mut")
                    nc.vector.tensor_copy(gt_sb[:], g3[:])

                    # muR^T = G Ws^T + s_k bs^T  (K, C)
                    mur_ps = ps("mm2")
                    for cc in range(OC):
                        nc.tensor.matmul(mur_ps[:K, :].bitcast(f32),
                                         gt_sb[:, cc, :],
                                         wst_sb[:, cc, :],
                                         start=(cc == 0), stop=False)
                    nc.tensor.matmul(mur_ps[:K, :].bitcast(f32),
                                     sk_sb[:],
                                     bs_row[:],
                                     start=False, stop=True)
                    # muT = muR^T / ||row||_2
                    sq_sb = smalls.tile([K, C], f32, tag="kc")
                    nc.scalar.square(sq_sb[:], mur_ps[:K, :])
                    nrm = smalls.tile([K, 1], f32, tag="nrm")
                    nc.vector.tensor_reduce(nrm[:], sq_sb[:], axis=AX.X,
                                            op=ALU.add)
                    nc.scalar.activation(nrm[:], nrm[:], AF.Ln)
                    nc.scalar.activation(nrm[:], nrm[:], AF.Exp, scale=-0.5)
                    muT_sb = mutp.tile([K, C], f32r, tag="muT",
                                       name=f"muT{b}_{it}")
                    nc.vector.tensor_scalar(muT_sb[:], mur_ps[:K, :], nrm[:],
                                            None, ALU.mult)
                    if it < NUM_ITER - 1:
                        mun_ps = ps("mm")
                        m3 = mun_ps[:, :OC * K].rearrange(
                            "p (j k) -> p j k", k=K)
                        for ot in range(OC):
                            nc.tensor.transpose(
                                m3[:, ot, :],
                                muT_sb[:, ot * P:(ot + 1) * P].bitcast(f32),
                                idr[:K, :K])
                        mu_nat = munp.tile([P, OC, K], f32r, tag="munat")
                        nc.vector.tensor_copy(mu_nat[:], m3[:])
                muT_of[b] = muT_sb

        # ================= L3: y2 / head / BN / final ====================
        with ExitStack() as l3:
            hwp = l3.enter_context(tc.tile_pool(name="hwp", bufs=1))
            psum3 = l3.enter_context(tc.tile_pool(name="psum3", bufs=1,
                                                  space="PSUM"))

            def ps(nm, bufs_tag=None):
                return psum3.tile([P, 512], f32, tag=bufs_tag or nm, name=nm)
            ztpool = l3.enter_context(tc.tile_pool(name="ztpool", bufs=2))
            ry2pool = l3.enter_context(tc.tile_pool(name="ry2pool", bufs=2))
            hstage = l3.enter_context(tc.tile_pool(name="hstage", bufs=4))
            hbig = l3.enter_context(tc.tile_pool(name="hbig", bufs=1))
            fstage = l3.enter_context(tc.tile_pool(name="fstage", bufs=3))
            fload = l3.enter_context(tc.tile_pool(name="fload", bufs=8))

            hwt_sb = hwp.tile([P, OC, C], f32r)  # Hw^T rows (o_part, oc, o2)
            nc.sync.dma_start(hwt_sb[:],
                              hwt_d.rearrange("(t p) c -> p t c", p=P)
                              .bitcast(f32r))
            h1_sb = hbig.tile([P, OC, N], bf16)
            h0_sb = hbig.tile([P, OC, N], bf16)
            h_of = [h0_sb, h1_sb]

            for b in range(BPC):
                z_sb = z_of[b]
                muT_sb = muT_of[b]
                for ch in range(NCH):
                    zt_ps = ps("ztr", "ztr%d" % (ch % 2))
                    z4 = zt_ps[:, :4 * P].rearrange("p (j q) -> p j q", q=P)
                    for j in range(4):
                        nc.tensor.transpose(
                            z4[:K, j, :],
                            z_sb[:, ch * 4 + j, :].bitcast(f32),
                            idr)
                    zt_sb = ztpool.tile([K, 512], f32r, tag="zT")
                    nc.vector.tensor_copy(zt_sb[:], zt_ps[:K, :4 * P])
                    ry2 = ry2pool.tile([P, OC, 512], f32r, tag="ry2")
                    for ot in range(OC):
                        y2_ps = ps("y2", "y2%d" % (ot % 2))
                        nc.tensor.matmul(y2_ps[:].bitcast(f32),
                                         muT_sb[:, ot * P:(ot + 1) * P],
                                         zt_sb[:],
                                         start=True, stop=True)
                        nc.vector.tensor_scalar(ry2[:, ot, :], y2_ps[:],
                                                0.0, None, ALU.max)
                    for o2 in range(OC):
                        h_ps = ps("h", "h%d" % (o2 % 2))
                        for oc in range(OC):
                            nc.tensor.matmul(
                                h_ps[:].bitcast(f32),
                                hwt_sb[:, oc, o2 * P:(o2 + 1) * P],
                                ry2[:, oc, :],
                                start=(oc == 0), stop=(oc == OC - 1))
                        acol = b * NCH + ch
                        dap = h_of[b][:, o2, ch * 512:(ch + 1) * 512]
                        if o2 % 2 == 0:
                            nc.vector.tensor_scalar(
                                dap, h_ps[:], 0.0, 0.0, ALU.add, ALU.add,
                                accum_out=sum_acc[:, o2, acol:acol + 1])
                        else:
                            nc.scalar.activation(
                                dap, h_ps[:], AF.Copy,
                                accum_out=sum_acc[:, o2, acol:acol + 1])
                        junk = hstage.tile([P, 512], bf16, tag="junk")
                        nc.scalar.activation(
                            junk[:], h_ps[:], AF.Square,
                            accum_out=sq_acc[:, o2, acol:acol + 1])

            # ---- prefetch first final-pass chunks (no dep on stats) ----
            FCH = N // 1024
            forder = [(b, o2, fc) for b in range(BPC)
                      for o2 in range(OC) for fc in range(FCH)]
            fql = []
            for (b, o2, fc) in forder[:8]:
                xr = fload.tile([P, 1024], f32, tag="xr")
                nc.sync.dma_start(
                    xr[:], x_d[b, o2 * P:(o2 + 1) * P,
                               fc * 1024:(fc + 1) * 1024])
                fql.append(xr)

            # ---- BN stats: aggregate, AllReduce, affine coefficients ----
            pack = statp.tile([P, 2 * OC], f32)
            packv = pack[:].rearrange("p (o two) -> p o two", two=2)
            nc.vector.tensor_reduce(packv[:, :, 0:1], sum_acc[:], axis=AX.X,
                                    op=ALU.add)
            nc.vector.tensor_reduce(packv[:, :, 1:2], sq_acc[:], axis=AX.X,
                                    op=ALU.add)
            nc.sync.dma_start(st_in_d[:], pack[:])
            if use_collective:
                nc.gpsimd.collective_compute(
                    "AllReduce", ALU.add,
                    replica_groups=[list(range(n_devices))],
                    ins=[st_in_d[:]],
                    outs=[st_out_d[:]],
                )
            else:
                nc.sync.dma_start(st_out_d[:], st_in_d[:])
            red = statp.tile([P, 2 * OC], f32)
            nc.sync.dma_start(red[:], st_out_d[:])
            a_sb = statp.tile([P, OC], f32)
            b2_sb = statp.tile([P, OC], f32)
            inv_nb = 1.0 / float(B * N)
            for o2 in range(OC):
                mean = smalls.tile([P, 1], f32, tag="mean")
                nc.vector.tensor_scalar(mean[:], red[:, 2 * o2:2 * o2 + 1],
                                        inv_nb, None, ALU.mult)
                var = smalls.tile([P, 1], f32, tag="var")
                nc.vector.tensor_scalar(var[:],
                                        red[:, 2 * o2 + 1:2 * o2 + 2],
                                        inv_nb, None, ALU.mult)
                msq = smalls.tile([P, 1], f32, tag="msq")
                nc.vector.tensor_tensor(msq[:], mean[:], mean[:], ALU.mult)
                nc.vector.tensor_tensor(var[:], var[:], msq[:], ALU.subtract)
                # rstd = exp(-0.5*ln(var+eps))
                nc.scalar.activation(var[:], var[:], AF.Ln, bias=eps_sb[:])
                nc.scalar.activation(var[:], var[:], AF.Exp, scale=-0.5)
                nc.vector.tensor_tensor(a_sb[:, o2:o2 + 1],
                                        gm_sb[:, o2:o2 + 1], var[:],
                                        ALU.mult)
                nc.vector.tensor_tensor(msq[:], mean[:], a_sb[:, o2:o2 + 1],
                                        ALU.mult)
                nc.vector.tensor_tensor(b2_sb[:, o2:o2 + 1],
                                        bt_sb[:, o2:o2 + 1], msq[:],
                                        ALU.subtract)

            # ---- final: out = relu(h*a + b2 + x) ----
            for fi, (b, o2, fc) in enumerate(forder):
                if fi < len(fql):
                    xr = fql[fi]
                else:
                    xr = fload.tile([P, 1024], f32, tag="xr")
                    nc.sync.dma_start(
                        xr[:], x_d[b, o2 * P:(o2 + 1) * P,
                                   fc * 1024:(fc + 1) * 1024])
                hap = h_of[b][:, o2, fc * 1024:(fc + 1) * 1024]
                t1 = fstage.tile([P, 1024], f32, tag="t1")
                nc.vector.scalar_tensor_tensor(
                    t1[:], hap, a_sb[:, o2:o2 + 1], xr[:],
                    ALU.mult, ALU.add)
                otile = fstage.tile([P, 1024], f32, tag="ot")
                nc.vector.tensor_scalar(otile[:], t1[:],
                                        b2_sb[:, o2:o2 + 1], 0.0,
                                        ALU.add, ALU.max)
                nc.sync.dma_start(
                    out_d[b, o2 * P:(o2 + 1) * P,
                          fc * 1024:(fc + 1) * 1024], otile[:])

    _hoist_extra_waits(nc)
    return nc


_ENGINE_SEM_PREFIX = {
    "EngineType.PE": "PE_",
    "EngineType.Activation": "Activation_",
    "EngineType.DVE": "DVE_",
    "EngineType.Pool": "Pool_",
    "EngineType.SP": "SP_",
}


def _hoist_extra_waits(nc):
    """This walrus build rejects compute-engine instructions carrying more
    than one sync wait. Engine queues are strict FIFO, so (a) an
    instruction waiting on its own engine's semaphore is always already
    satisfied -> drop it; (b) any extra waits can be hoisted onto NoOp
    instructions injected just before, one wait each -- identical
    semantics."""
    import concourse.mybir as mybir
    nid = 0
    for blk in nc.m.functions[0].blocks:
        out = []
        changed = False
        for i in blk.instructions:
            si = getattr(i, "sync_info", None)
            eng = str(getattr(i, "engine", None))
            waits = list(si.on_wait) if si and si.on_wait else []
            if len(waits) > 1 and eng in _ENGINE_SEM_PREFIX:
                selfp = _ENGINE_SEM_PREFIX[eng]
                waits = [w for w in waits if not w.ant_name.startswith(selfp)]
                for w in waits[:-1]:
                    nid += 1
                    out.append(mybir.InstNoOp(
                        name=f"I-waitnop-{nid}",
                        engine=i.engine,
                        sync_info=mybir.SyncInfo(on_wait=[w], on_update=[]),
                        bass_nofuse=True,
                    ))
                i.sync_info = mybir.SyncInfo(
                    on_wait=waits[-1:], on_update=list(si.on_update or []))
                changed = True
            out.append(i)
        if changed:
            blk.instructions = out


def get_nc():
    if "nc" not in _cache:
        _cache["nc"] = _build_nc()
    return _cache["nc"]


def run(inputs_by_core, trace=False):
    from concourse.bass_utils import run_bass_kernel_spmd
    nc = get_nc()
    return run_bass_kernel_spmd(nc, inputs_by_core, list(range(NCORES)),
                                trace=trace)


def make_in_maps(x, mu, stem_w, stem_b, head_w, head_b, bn_gamma, bn_beta):
    x = np.ascontiguousarray(np.asarray(x, np.float32)).reshape(B, C, N)
    common = {
        "mu": np.ascontiguousarray(np.asarray(mu, np.float32)),
        "ws": np.ascontiguousarray(np.asarray(stem_w, np.float32)),
        "wst": np.ascontiguousarray(np.asarray(stem_w, np.float32).T),
        "hwt": np.ascontiguousarray(np.asarray(head_w, np.float32).T),
        "bs": np.ascontiguousarray(
            np.stack([np.asarray(stem_b, np.float32),
                      np.zeros(C, np.float32)], axis=1)),
        "gm": np.ascontiguousarray(np.asarray(bn_gamma, np.float32)),
        "onec": np.ones(128, np.float32),
        "bt": np.ascontiguousarray(np.asarray(bn_beta, np.float32)),
    }
    return [
        {"x": np.ascontiguousarray(x[i * BPC:(i + 1) * BPC]), **common}
        for i in range(NCORES)
    ]


def kernel(x, mu, stem_w, stem_b, head_w, head_b, bn_gamma, bn_beta):
    in_maps = make_in_maps(x, mu, stem_w, stem_b, head_w, head_b,
                           bn_gamma, bn_beta)
    res = run(in_maps, trace=False)
    out = np.concatenate([res.results[i]["out"] for i in range(NCORES)],
                         axis=0)
    return out.reshape(B, C, 64, 64).astype(np.float32)

